# revision 1
# baseline (speedup 1.0000x reference)
"""Trainium2 Bass kernel for KerasCrossAttentionModule (B=8, S=4096, D=256).

Sharding: data-parallel over batch B across 8 NeuronCores (1 batch/core).
Per-core math (all on device):
    qT = queries[b] + q_posT          # (D, S) fp32 -> fp16
    kT = keys[b]    + k_posT          # (D, S) fp32 -> fp16
    v  = values[b].T                  # (S, D) host-transposed layout, cast fp16
    scoresT[j, i] = sum_d kT[d, j] * qT[d, i]        (PE, fp16 in / fp32 acc)
    E = exp(scale * scoresT)                          (ACT, fp32 -> fp16)
    denom[i] = sum_j E[j, i]                          (DVE partition tree)
    outT[d, i] = sum_j v[j, d] * E[j, i] / denom[i]   (PE + DVE)
Output DRAM tensor is (D, S) == (DV, H*W), which is exactly the reference
output layout per batch, so no final transpose is needed.
"""

import os
import sys

import numpy as np

for _p in ("/opt/trn_rl_repo", "/root/.axon_site/_ro/trn_rl_repo"):
    if os.path.isdir(_p) and _p not in sys.path:
        sys.path.insert(0, _p)

import concourse.bass as bass
from concourse import bacc
import concourse.tile as tile
from concourse import mybir
from concourse.bass_utils import run_bass_kernel_spmd

B = 8
D = 256
S = 4096
HALF = 128            # partition-dim tile of D
N_DH = D // HALF      # 2 halves of the head dim
SCALE = float(D) ** -0.5

FP32 = mybir.dt.float32
FP16 = mybir.dt.float16

# Set by test harness to capture a profile; harness-default is plain run.
TRACE = False
LAST_RESULT = None


def _build_attention(s=S, qsb=1024, qc=512):
    """One-core program; identical on all 8 cores (pure data parallel)."""
    nj = s // HALF        # key chunks (partition dim of scoresT)
    n_sb = s // qsb       # query super-blocks
    n_qc = qsb // qc      # matmul free-dim chunks per super-block

    nc = bacc.Bacc("TRN2")
    qt = nc.dram_tensor("qt", [D, s], FP32, kind="ExternalInput")
    kt = nc.dram_tensor("kt", [D, s], FP32, kind="ExternalInput")
    vt = nc.dram_tensor("vt", [s, D], FP32, kind="ExternalInput")
    qpt = nc.dram_tensor("qpt", [D, s], FP32, kind="ExternalInput")
    kpt = nc.dram_tensor("kpt", [D, s], FP32, kind="ExternalInput")
    out = nc.dram_tensor("out", [D, s], FP32, kind="ExternalOutput")

    with tile.TileContext(nc) as tc:
        with (
            tc.tile_pool(name="prep", bufs=4) as prep,
            tc.tile_pool(name="qk16", bufs=1) as qk16,
            tc.tile_pool(name="vpool", bufs=1) as vpool,
            tc.tile_pool(name="expp", bufs=8) as expp,
            tc.tile_pool(name="daccp", bufs=2) as daccp,
            tc.tile_pool(name="onorm", bufs=2) as onorm,
            tc.tile_pool(name="ps_s", bufs=2, space="PSUM") as ps_s,
            tc.tile_pool(name="ps_av", bufs=1, space="PSUM") as ps_av,
        ):
            # ---- prep: add pos embeddings, cast to fp16 ----------------
            qh = [qk16.tile([HALF, s], FP16, tag=f"qh{dh}", name=f"qh{dh}")
                  for dh in range(N_DH)]
            kh = [qk16.tile([HALF, s], FP16, tag=f"kh{dh}", name=f"kh{dh}")
                  for dh in range(N_DH)]
            # SWDGE DMAs cast fp32->fp16 in flight; the second DMA per chunk
            # accumulates (dst += src), so no engine add is needed at all.
            n_pc = s // 512
            for c in range(n_pc):
                cs = slice(c * 512, (c + 1) * 512)
                for dh in range(N_DH):
                    ds_ = slice(dh * HALF, (dh + 1) * HALF)
                    nc.gpsimd.dma_start(out=kh[dh][:, cs], in_=kt[ds_, cs])
                    nc.gpsimd.dma_start(out=kh[dh][:, cs], in_=kpt[ds_, cs],
                                        accum_op=mybir.AluOpType.add)
                    nc.gpsimd.dma_start(out=qh[dh][:, cs], in_=qt[ds_, cs])
                    nc.gpsimd.dma_start(out=qh[dh][:, cs], in_=qpt[ds_, cs],
                                        accum_op=mybir.AluOpType.add)

            # ---- constants --------------------------------------------
            ones_h = qk16.tile([HALF, 1], FP16, tag="ones_h", name="ones_h")
            nc.vector.memset(ones_h, 1.0)
            ones_b = qk16.tile([1, HALF], FP32, tag="ones_b", name="ones_b")
            nc.vector.memset(ones_b, 1.0)

            # ---- v: load (S, D) chunks, cast to fp16 -------------------
            vh = []
            for j in range(nj):
                vhj = vpool.tile([HALF, D], FP16, tag=f"vh{j}", name=f"vh{j}")
                nc.gpsimd.dma_start(out=vhj, in_=vt[j * HALF:(j + 1) * HALF, :])
                vh.append(vhj)

            # ---- main attention loop -----------------------------------
            for sb in range(n_sb):
                qs0 = sb * qsb
                av = [ps_av.tile([HALF, qsb], FP32, tag=f"av{dh}",
                                 name=f"av{dh}") for dh in range(N_DH)]
                dacc = daccp.tile([HALF, qsb], FP16, tag="dacc", name="dacc")
                for j in range(nj):
                    js = slice(j * HALF, (j + 1) * HALF)
                    sp = ps_s.tile([HALF, qsb], FP32, tag="sp", name="sp")
                    for dh in range(N_DH):
                        for c in range(n_qc):
                            nc.tensor.matmul(
                                sp[:, c * qc:(c + 1) * qc],
                                kh[dh][:, js],
                                qh[dh][:, qs0 + c * qc:qs0 + (c + 1) * qc],
                                start=(dh == 0),
                                stop=(dh == N_DH - 1),
                            )
                    et = expp.tile([HALF, qsb], FP16, tag="et", name="et")
                    nc.scalar.activation(
                        et, sp, mybir.ActivationFunctionType.Exp, scale=SCALE
                    )
                    if j == 0:
                        nc.vector.tensor_copy(dacc, et)
                    else:
                        nc.vector.tensor_add(dacc, dacc, et)
                    for dh in range(N_DH):
                        for c in range(n_qc):
                            nc.tensor.matmul(
                                av[dh][:, c * qc:(c + 1) * qc],
                                vh[j][:, dh * HALF:(dh + 1) * HALF],
                                et[:, c * qc:(c + 1) * qc],
                                start=(j == 0),
                                stop=(j == nj - 1),
                            )
                # denominator: partition-reduce via ones matmul (M=1)
                dred = ps_s.tile([1, qsb], FP32, tag="sp", name="dred")
                for c in range(n_qc):
                    nc.tensor.matmul(
                        dred[:, c * qc:(c + 1) * qc],
                        ones_h,
                        dacc[:, c * qc:(c + 1) * qc],
                        start=True,
                        stop=True,
                    )
                dr = onorm.tile([1, qsb], FP32, tag="dr", name="dr")
                nc.vector.reciprocal(dr, dred)
                # broadcast 1/denom across partitions via K=1 matmul
                rb = ps_s.tile([HALF, qsb], FP32, tag="sp", name="rb")
                for c in range(n_qc):
                    nc.tensor.matmul(
                        rb[:, c * qc:(c + 1) * qc],
                        ones_b,
                        dr[:, c * qc:(c + 1) * qc],
                        start=True,
                        stop=True,
                    )
                rbs = onorm.tile([HALF, qsb], FP32, tag="rbs", name="rbs")
                nc.vector.tensor_copy(rbs, rb)
                for dh in range(N_DH):
                    ot = onorm.tile([HALF, qsb], FP32, tag="ot", name="ot",
                                    bufs=4)
                    nc.vector.tensor_mul(ot, av[dh], rbs)
                    nc.sync.dma_start(
                        out=out[dh * HALF:(dh + 1) * HALF,
                                qs0:qs0 + qsb],
                        in_=ot,
                    )
    nc.finalize()
    return nc


_NC_CACHE = {}


def _get_program():
    if "nc" not in _NC_CACHE:
        _NC_CACHE["nc"] = _build_attention()
    return _NC_CACHE["nc"]


def kernel(queries, keys, values, q_pos, k_pos):
    global LAST_RESULT
    q = np.asarray(queries, dtype=np.float32).reshape(B, D, S)
    k = np.asarray(keys, dtype=np.float32).reshape(B, D, S)
    v = np.asarray(values, dtype=np.float32).reshape(B, D, S)
    v_t = np.ascontiguousarray(v.transpose(0, 2, 1))          # (B, S, D)
    qpt = np.ascontiguousarray(np.asarray(q_pos, np.float32).reshape(S, D).T)
    kpt = np.ascontiguousarray(np.asarray(k_pos, np.float32).reshape(S, D).T)

    nc = _get_program()
    in_maps = [
        {
            "qt": np.ascontiguousarray(q[b]),
            "kt": np.ascontiguousarray(k[b]),
            "vt": v_t[b],
            "qpt": qpt,
            "kpt": kpt,
        }
        for b in range(B)
    ]
    res = run_bass_kernel_spmd(nc, in_maps, list(range(B)), trace=TRACE)
    LAST_RESULT = res
    out = np.stack([res.results[b]["out"] for b in range(B)])  # (B, D, S)
    return out.reshape(B, D, 64, 64).astype(np.float32)



# revision 2
# speedup vs baseline: 1.2752x; 1.2752x over previous
"""Trainium2 Bass kernel for KerasCrossAttentionModule (B=8, S=4096, D=256).

Sharding: data-parallel over batch B across 8 NeuronCores (1 batch/core).

Host prep (cheap, O(B*S*D)): fold the positional embeddings into q/k
(q + q_pos, k + k_pos), transpose to (D, S), cast fp16; rearrange v to a
key-major layout (128, 32*256) so the whole tensor is one SBUF tile.

Per-core device math (all engines pipelined):
    scoresT[j*128+p, i] = sum_d kh[d, jp] * qh[d, i]     (PE, fp16/fp32 acc)
    E = exp(scale * scoresT)                             (ACT, fp32 -> fp16)
    dacc[p, i] += E[jp, i]                               (DVE partial rowsum)
    av[d, i]  += sum_jp vr[jp, d] * E[jp, i]             (PE accum over j)
    denom = ones^T @ dacc   (M=128 -> full-width, no broadcast needed)
    out[d, i] = av[d, i] * (1 / denom[i])                (DVE approx recip)

The PE instruction stream is software-pipelined: the score matmuls for
iteration i+2 are issued before the AV matmuls of iteration i, so the
exp() latency never stalls the tensor engine.  Inputs stream over the two
HWDGE queues (sync + scalar) in consumption order while a short warm-up
matmul burst keeps the PE HAM clock-gate at full rate.

Output DRAM tensor is (D, S) == (DV, H*W), exactly the reference output
layout per batch, so no final transpose is needed.
"""

import os
import sys

import numpy as np

for _p in ("/opt/trn_rl_repo", "/root/.axon_site/_ro/trn_rl_repo"):
    if os.path.isdir(_p) and _p not in sys.path:
        sys.path.insert(0, _p)

import concourse.bass as bass
from concourse import bacc
import concourse.tile as tile
from concourse import mybir
from concourse.bass_utils import run_bass_kernel_spmd

B = 8
D = 256
S = 4096
HALF = 128            # partition-dim tile (D halves / key chunks)
N_DH = D // HALF      # 2 halves of the head dim
QSB = 512             # query super-block (PSUM-bank sized)
N_SB = S // QSB       # 8 query super-blocks
NJ = S // HALF        # 32 key chunks
SCALE = float(D) ** -0.5
N_WARM = 36           # PE warm-up matmuls (~3.7us @ cold clock)

FP32 = mybir.dt.float32
FP16 = mybir.dt.float16

# Set by test harness to capture a profile; harness-default is plain run.
TRACE = False
LAST_RESULT = None


def _build_attention():
    """One-core program; identical on all 8 cores (pure data parallel)."""
    nc = bacc.Bacc("TRN2")
    qh_d = nc.dram_tensor("qh", [D, S], FP16, kind="ExternalInput")
    kh_d = nc.dram_tensor("kh", [D, S], FP16, kind="ExternalInput")
    vr_d = nc.dram_tensor("vr", [HALF, NJ * D], FP16, kind="ExternalInput")
    out_d = nc.dram_tensor("out", [D, S], FP32, kind="ExternalOutput")

    with tile.TileContext(nc) as tc:
        with (
            tc.tile_pool(name="big", bufs=1) as big,
            tc.tile_pool(name="expp", bufs=16) as expp,
            tc.tile_pool(name="daccp", bufs=2) as daccp,
            tc.tile_pool(name="rbsp", bufs=2) as rbsp,
            tc.tile_pool(name="otp", bufs=4) as otp,
            tc.tile_pool(name="ps_s", bufs=3, space="PSUM") as ps_s,
            tc.tile_pool(name="ps_av", bufs=2, space="PSUM") as ps_av,
            tc.tile_pool(name="ps_d", bufs=1, space="PSUM") as ps_d,
        ):
            qh = [big.tile([HALF, S], FP16, tag=f"qh{dh}", name=f"qh{dh}")
                  for dh in range(N_DH)]
            kh = [big.tile([HALF, S], FP16, tag=f"kh{dh}", name=f"kh{dh}")
                  for dh in range(N_DH)]
            vr = big.tile([HALF, NJ * D], FP16, tag="vr", name="vr")
            ones_h = big.tile([HALF, HALF], FP16, tag="ones_h", name="ones_h")
            nc.vector.memset(ones_h, 1.0)

            # ---- PE warm-up: flip the HAM clock gate to 8/8 while the
            # first input chunks stream in. ---------------------------------
            wt = ps_d.tile([HALF, QSB], FP32, tag="dred", name="warm")
            for _ in range(N_WARM):
                nc.tensor.matmul(wt[:, :HALF], ones_h, ones_h,
                                 start=True, stop=True)

            # ---- input DMAs, consumption order, two HWDGE queues ----------
            # sync queue: k (all of it is consumed within the first sb) and
            # the first q super-block.
            for dh in range(N_DH):
                ds_ = slice(dh * HALF, (dh + 1) * HALF)
                nc.sync.dma_start(out=kh[dh][:, 0:512], in_=kh_d[ds_, 0:512])
            for dh in range(N_DH):
                ds_ = slice(dh * HALF, (dh + 1) * HALF)
                nc.sync.dma_start(out=qh[dh][:, 0:512], in_=qh_d[ds_, 0:512])
            for a, b in ((512, 1536), (1536, 2560), (2560, 3584), (3584, 4096)):
                for dh in range(N_DH):
                    ds_ = slice(dh * HALF, (dh + 1) * HALF)
                    nc.sync.dma_start(out=kh[dh][:, a:b], in_=kh_d[ds_, a:b])
            # scalar queue: v chunks (consumed from j=0) then the rest of q.
            vchunks = [(0, 512), (512, 1024)] + [
                (1024 * i, 1024 * (i + 1)) for i in range(1, 8)
            ]
            for a, b in vchunks:
                nc.scalar.dma_start(out=vr[:, a:b], in_=vr_d[:, a:b])
            for a, b in ((512, 1536), (1536, 2560), (2560, 3584), (3584, 4096)):
                for dh in range(N_DH):
                    ds_ = slice(dh * HALF, (dh + 1) * HALF)
                    nc.scalar.dma_start(out=qh[dh][:, a:b], in_=qh_d[ds_, a:b])

            # ---- software-pipelined main loop ------------------------------
            ets = {}
            daccs = {}
            avs = {}

            def do_s(i):
                sb, j = divmod(i, NJ)
                qs = slice(sb * QSB, (sb + 1) * QSB)
                js = slice(j * HALF, (j + 1) * HALF)
                sp = ps_s.tile([HALF, QSB], FP32, tag="sp", name="sp")
                nc.tensor.matmul(sp, kh[0][:, js], qh[0][:, qs],
                                 start=True, stop=False)
                nc.tensor.matmul(sp, kh[1][:, js], qh[1][:, qs],
                                 start=False, stop=True)
                et = expp.tile([HALF, QSB], FP16, tag="et", name="et")
                nc.scalar.activation(
                    et, sp, mybir.ActivationFunctionType.Exp, scale=SCALE
                )
                if j == 0:
                    dacc = daccp.tile([HALF, QSB], FP16, tag="dacc",
                                      name="dacc")
                    nc.vector.tensor_copy(dacc, et)
                    daccs[sb] = dacc
                else:
                    nc.vector.tensor_add(daccs[sb], daccs[sb], et)
                ets[i] = et

            def do_av(i):
                sb, j = divmod(i, NJ)
                if j == 0:
                    avs[sb] = [
                        ps_av.tile([HALF, QSB], FP32, tag=f"av{dh}",
                                   name=f"av{dh}")
                        for dh in range(N_DH)
                    ]
                av = avs[sb]
                et = ets.pop(i)
                for dh in range(N_DH):
                    vs = slice(j * D + dh * HALF, j * D + (dh + 1) * HALF)
                    nc.tensor.matmul(av[dh], vr[:, vs], et,
                                     start=(j == 0), stop=(j == NJ - 1))
                if j == NJ - 1:
                    dacc = daccs.pop(sb)
                    dredt = ps_d.tile([HALF, QSB], FP32, tag="dred",
                                      name="dred")
                    nc.tensor.matmul(dredt, ones_h, dacc,
                                     start=True, stop=True)
                    rbs = rbsp.tile([HALF, QSB], FP32, tag="rbs", name="rbs")
                    nc.vector.reciprocal_approx_fast(out=rbs, in_=dredt)
                    for dh in range(N_DH):
                        ot = otp.tile([HALF, QSB], FP32, tag="ot", name="ot")
                        nc.vector.tensor_mul(ot, av[dh], rbs)
                        nc.sync.dma_start(
                            out=out_d[dh * HALF:(dh + 1) * HALF,
                                      sb * QSB:(sb + 1) * QSB],
                            in_=ot,
                        )
                    avs.pop(sb)

            n_it = N_SB * NJ
            do_s(0)
            do_s(1)
            for i in range(2, n_it):
                do_s(i)
                do_av(i - 2)
            do_av(n_it - 2)
            do_av(n_it - 1)
    nc.finalize()
    return nc


_NC_CACHE = {}


def _get_program():
    if "nc" not in _NC_CACHE:
        _NC_CACHE["nc"] = _build_attention()
    return _NC_CACHE["nc"]


def kernel(queries, keys, values, q_pos, k_pos):
    global LAST_RESULT
    q = np.asarray(queries, dtype=np.float32).reshape(B, D, S)
    k = np.asarray(keys, dtype=np.float32).reshape(B, D, S)
    v = np.asarray(values, dtype=np.float32).reshape(B, D, S)
    qpt = np.asarray(q_pos, np.float32).reshape(S, D).T       # (D, S)
    kpt = np.asarray(k_pos, np.float32).reshape(S, D).T
    qh = (q + qpt[None]).astype(np.float16)                   # (B, D, S)
    kh = (k + kpt[None]).astype(np.float16)
    # v (B, D, S) -> (B, 128, NJ*D): vr[b, p, j*D + d] = v[b, d, j*128 + p]
    vr = np.ascontiguousarray(
        v.reshape(B, D, NJ, HALF).transpose(0, 3, 2, 1).reshape(B, HALF, NJ * D)
    ).astype(np.float16)

    nc = _get_program()
    in_maps = [
        {
            "qh": np.ascontiguousarray(qh[b]),
            "kh": np.ascontiguousarray(kh[b]),
            "vr": vr[b],
        }
        for b in range(B)
    ]
    res = run_bass_kernel_spmd(nc, in_maps, list(range(B)), trace=TRACE)
    LAST_RESULT = res
    out = np.stack([res.results[b]["out"] for b in range(B)])  # (B, D, S)
    return out.reshape(B, D, 64, 64).astype(np.float32)


# revision 3
# speedup vs baseline: 1.5271x; 1.1975x over previous
"""Trainium2 Bass kernel for KerasCrossAttentionModule (B=8, S=4096, D=256).

Sharding: data-parallel over batch B across 8 NeuronCores (1 batch/core).

Host prep (cheap, O(B*S*D)): fold the positional embeddings into q/k
(q + q_pos, k + k_pos), transpose to (D, S), cast fp16; rearrange v to a
key-major layout (128, 32*256) so the whole tensor is one SBUF tile.

Per-core device math (all engines pipelined):
    scoresT[j*128+p, i] = sum_d kh[d, jp] * qh[d, i]     (PE, fp16/fp32 acc)
    E = exp(scale * scoresT)                             (ACT, fp32 -> fp16)
    dacc[p, i] += E[jp, i]                               (DVE partial rowsum)
    av[d, i]  += sum_jp vr[jp, d] * E[jp, i]             (PE accum over j)
    denom = ones^T @ dacc   (M=128 -> full-width, no broadcast needed)
    out[d, i] = av[d, i] * (1 / denom[i])                (DVE approx recip)

The PE instruction stream is software-pipelined: the score matmuls for
iteration i+2 are issued before the AV matmuls of iteration i, so the
exp() latency never stalls the tensor engine.  Inputs stream over the
sync HWDGE queue and the gpsimd SWDGE queue in consumption order (the
ACT queue carries only activations - DMA triggers on it would delay the
first exp by ~20us).  A short warm-up matmul burst flips the PE HAM
clock-gate to full rate while the first input chunks land.

Output DRAM tensor is (D, S) == (DV, H*W), exactly the reference output
layout per batch, so no final transpose is needed.
"""

import os
import sys

import numpy as np

for _p in ("/opt/trn_rl_repo", "/root/.axon_site/_ro/trn_rl_repo"):
    if os.path.isdir(_p) and _p not in sys.path:
        sys.path.insert(0, _p)

import concourse.bass as bass
from concourse import bacc
import concourse.tile as tile
from concourse import mybir
from concourse.bass_utils import run_bass_kernel_spmd

B = 8
D = 256
S = 4096
HALF = 128            # partition-dim tile (D halves / key chunks)
N_DH = D // HALF      # 2 halves of the head dim
QSB = 1024            # query super-block (2 PSUM banks)
QC = 512              # matmul free-dim chunk (1 PSUM bank)
N_QC = QSB // QC
N_SB = S // QSB       # 4 query super-blocks
NJ = S // HALF        # 32 key chunks
SCALE = float(D) ** -0.5
N_WARM = 36           # PE warm-up matmuls (~3.9us @ cold clock)

FP32 = mybir.dt.float32
FP16 = mybir.dt.float16

# Set by test harness to capture a profile; harness-default is plain run.
TRACE = False
LAST_RESULT = None


def _build_attention():
    """One-core program; identical on all 8 cores (pure data parallel)."""
    nc = bacc.Bacc("TRN2")
    qh_d = nc.dram_tensor("qh", [D, S], FP16, kind="ExternalInput")
    kh_d = nc.dram_tensor("kh", [D, S], FP16, kind="ExternalInput")
    vr_d = nc.dram_tensor("vr", [HALF, NJ * D], FP16, kind="ExternalInput")
    out_d = nc.dram_tensor("out", [D, S], FP32, kind="ExternalOutput")

    with tile.TileContext(nc) as tc:
        with (
            tc.tile_pool(name="big", bufs=1) as big,
            tc.tile_pool(name="expp", bufs=8) as expp,
            tc.tile_pool(name="daccp", bufs=2) as daccp,
            tc.tile_pool(name="rbsp", bufs=2) as rbsp,
            tc.tile_pool(name="otp", bufs=4) as otp,
            tc.tile_pool(name="ps_s", bufs=2, space="PSUM") as ps_s,
            tc.tile_pool(name="ps_av", bufs=1, space="PSUM") as ps_av,
        ):
            qh = [big.tile([HALF, S], FP16, tag=f"qh{dh}", name=f"qh{dh}")
                  for dh in range(N_DH)]
            kh = [big.tile([HALF, S], FP16, tag=f"kh{dh}", name=f"kh{dh}")
                  for dh in range(N_DH)]
            vr = big.tile([HALF, NJ * D], FP16, tag="vr", name="vr")
            ones_h = big.tile([HALF, HALF], FP16, tag="ones_h", name="ones_h")
            nc.vector.memset(ones_h, 1.0)

            # ---- PE warm-up: flip the HAM clock gate to 8/8 while the
            # first input chunks stream in. ---------------------------------
            wt = ps_s.tile([HALF, QSB], FP32, tag="sp", name="warm")
            for _ in range(N_WARM):
                nc.tensor.matmul(wt[:, :HALF], ones_h, ones_h,
                                 start=True, stop=True)

            # ---- input DMAs, consumption order -----------------------------
            # sync (HWDGE) queue: k (all consumed within the first sb) and
            # the first q super-block; outputs are appended later.
            for dh in range(N_DH):
                ds_ = slice(dh * HALF, (dh + 1) * HALF)
                nc.sync.dma_start(out=kh[dh][:, 0:512], in_=kh_d[ds_, 0:512])
            for dh in range(N_DH):
                ds_ = slice(dh * HALF, (dh + 1) * HALF)
                nc.sync.dma_start(out=qh[dh][:, 0:1024],
                                  in_=qh_d[ds_, 0:1024])
            for a, b in ((512, 1536), (1536, 2560), (2560, 3584), (3584, 4096)):
                for dh in range(N_DH):
                    ds_ = slice(dh * HALF, (dh + 1) * HALF)
                    nc.sync.dma_start(out=kh[dh][:, a:b], in_=kh_d[ds_, a:b])
            # gpsimd (SWDGE) queue: v chunks (consumed from j=0), then the
            # remaining q super-blocks.
            vchunks = [(0, 512), (512, 1024)] + [
                (1024 * i, 1024 * (i + 1)) for i in range(1, 8)
            ]
            for a, b in vchunks:
                nc.gpsimd.dma_start(out=vr[:, a:b], in_=vr_d[:, a:b])
            for sb in range(1, N_SB):
                for dh in range(N_DH):
                    ds_ = slice(dh * HALF, (dh + 1) * HALF)
                    cs = slice(sb * QSB, (sb + 1) * QSB)
                    nc.gpsimd.dma_start(out=qh[dh][:, cs], in_=qh_d[ds_, cs])

            # ---- software-pipelined main loop ------------------------------
            ets = {}
            daccs = {}
            avs = {}

            def do_s(i):
                sb, j = divmod(i, NJ)
                js = slice(j * HALF, (j + 1) * HALF)
                sp = ps_s.tile([HALF, QSB], FP32, tag="sp", name="sp")
                for c in range(N_QC):
                    cs = slice(sb * QSB + c * QC, sb * QSB + (c + 1) * QC)
                    for dh in range(N_DH):
                        nc.tensor.matmul(sp[:, c * QC:(c + 1) * QC],
                                         kh[dh][:, js], qh[dh][:, cs],
                                         start=(dh == 0), stop=(dh == 1))
                et = expp.tile([HALF, QSB], FP16, tag="et", name="et")
                nc.scalar.activation(
                    et, sp, mybir.ActivationFunctionType.Exp, scale=SCALE
                )
                if j == 0:
                    dacc = daccp.tile([HALF, QSB], FP16, tag="dacc",
                                      name="dacc")
                    nc.vector.tensor_copy(dacc, et)
                    daccs[sb] = dacc
                else:
                    nc.vector.tensor_add(daccs[sb], daccs[sb], et)
                ets[i] = et

            def do_av(i):
                sb, j = divmod(i, NJ)
                if j == 0:
                    avs[sb] = [
                        ps_av.tile([HALF, QSB], FP32, tag=f"av{dh}",
                                   name=f"av{dh}")
                        for dh in range(N_DH)
                    ]
                av = avs[sb]
                et = ets.pop(i)
                for dh in range(N_DH):
                    vs = slice(j * D + dh * HALF, j * D + (dh + 1) * HALF)
                    for c in range(N_QC):
                        nc.tensor.matmul(av[dh][:, c * QC:(c + 1) * QC],
                                         vr[:, vs],
                                         et[:, c * QC:(c + 1) * QC],
                                         start=(j == 0), stop=(j == NJ - 1))
                if j == NJ - 1:
                    dacc = daccs.pop(sb)
                    dredt = ps_s.tile([HALF, QSB], FP32, tag="sp",
                                      name="dred")
                    for c in range(N_QC):
                        nc.tensor.matmul(dredt[:, c * QC:(c + 1) * QC],
                                         ones_h,
                                         dacc[:, c * QC:(c + 1) * QC],
                                         start=True, stop=True)
                    rbs = rbsp.tile([HALF, QSB], FP32, tag="rbs", name="rbs")
                    nc.vector.reciprocal_approx_fast(out=rbs, in_=dredt)
                    for dh in range(N_DH):
                        ot = otp.tile([HALF, QSB], FP32, tag="ot", name="ot")
                        nc.vector.tensor_mul(ot, av[dh], rbs)
                        nc.sync.dma_start(
                            out=out_d[dh * HALF:(dh + 1) * HALF,
                                      sb * QSB:(sb + 1) * QSB],
                            in_=ot,
                        )
                    avs.pop(sb)

            n_it = N_SB * NJ
            do_s(0)
            do_s(1)
            for i in range(2, n_it):
                do_s(i)
                do_av(i - 2)
            do_av(n_it - 2)
            do_av(n_it - 1)
    nc.finalize()
    return nc


_NC_CACHE = {}


def _get_program():
    if "nc" not in _NC_CACHE:
        _NC_CACHE["nc"] = _build_attention()
    return _NC_CACHE["nc"]


def kernel(queries, keys, values, q_pos, k_pos):
    global LAST_RESULT
    q = np.asarray(queries, dtype=np.float32).reshape(B, D, S)
    k = np.asarray(keys, dtype=np.float32).reshape(B, D, S)
    v = np.asarray(values, dtype=np.float32).reshape(B, D, S)
    qpt = np.asarray(q_pos, np.float32).reshape(S, D).T       # (D, S)
    kpt = np.asarray(k_pos, np.float32).reshape(S, D).T
    qh = (q + qpt[None]).astype(np.float16)                   # (B, D, S)
    kh = (k + kpt[None]).astype(np.float16)
    # v (B, D, S) -> (B, 128, NJ*D): vr[b, p, j*D + d] = v[b, d, j*128 + p]
    vr = np.ascontiguousarray(
        v.reshape(B, D, NJ, HALF).transpose(0, 3, 2, 1).reshape(B, HALF, NJ * D)
    ).astype(np.float16)

    nc = _get_program()
    in_maps = [
        {
            "qh": np.ascontiguousarray(qh[b]),
            "kh": np.ascontiguousarray(kh[b]),
            "vr": vr[b],
        }
        for b in range(B)
    ]
    res = run_bass_kernel_spmd(nc, in_maps, list(range(B)), trace=TRACE)
    LAST_RESULT = res
    out = np.stack([res.results[b]["out"] for b in range(B)])  # (B, D, S)
    return out.reshape(B, D, 64, 64).astype(np.float32)


# revision 5
# speedup vs baseline: 1.5497x; 1.0149x over previous
"""Trainium2 Bass kernel for KerasCrossAttentionModule (B=8, S=4096, D=256).

Sharding: data-parallel over batch B across 8 NeuronCores (1 batch/core).

Host prep (cheap, O(B*S*D)): fold the positional embeddings into q/k
(q + q_pos, k + k_pos), transpose to (D, S), cast fp16; rearrange v to a
key-major layout (128, 32*256) so the whole tensor is one SBUF tile.

Per-core device math (all engines pipelined):
    scoresT[j*128+p, i] = sum_d kh[d, jp] * qh[d, i]     (PE, fp16/fp32 acc)
    E = exp(scale * scoresT)                             (ACT, fp32 -> fp16)
    dacc[p, i] += E[jp, i]                               (DVE partial rowsum)
    av[d, i]  += sum_jp vr[jp, d] * E[jp, i]             (PE accum over j)
    denom = ones^T @ dacc   (M=128 -> full-width, no broadcast needed)
    out[d, i] = av[d, i] * (1 / denom[i])                (DVE approx recip)

The PE instruction stream is software-pipelined: the score matmuls for
iteration i+2 are issued before the AV matmuls of iteration i, so the
exp() latency never stalls the tensor engine.  Inputs stream over the
sync HWDGE queue and the gpsimd SWDGE queue in consumption order (the
ACT queue carries only activations - DMA triggers on it would delay the
first exp by ~20us).  A short warm-up matmul burst flips the PE HAM
clock-gate to full rate while the first input chunks land.

Output DRAM tensor is (D, S) == (DV, H*W), exactly the reference output
layout per batch, so no final transpose is needed.
"""

import os
import sys

import numpy as np

for _p in ("/opt/trn_rl_repo", "/root/.axon_site/_ro/trn_rl_repo"):
    if os.path.isdir(_p) and _p not in sys.path:
        sys.path.insert(0, _p)

import concourse.bass as bass
from concourse import bacc
import concourse.tile as tile
from concourse import mybir
from concourse.bass_utils import run_bass_kernel_spmd

B = 8
D = 256
S = 4096
HALF = 128            # partition-dim tile (D halves / key chunks)
N_DH = D // HALF      # 2 halves of the head dim
QSB = 1024            # query super-block (2 PSUM banks)
QC = 512              # matmul free-dim chunk (1 PSUM bank)
N_QC = QSB // QC
N_SB = S // QSB       # 4 query super-blocks
NJ = S // HALF        # 32 key chunks
SCALE = float(D) ** -0.5
N_WARM = 36           # PE warm-up matmuls (~3.9us @ cold clock)

FP32 = mybir.dt.float32
FP16 = mybir.dt.float16

# Set by test harness to capture a profile; harness-default is plain run.
TRACE = False
LAST_RESULT = None


def _build_attention():
    """One-core program; identical on all 8 cores (pure data parallel)."""
    nc = bacc.Bacc("TRN2")
    qh_d = nc.dram_tensor("qh", [D, S], FP16, kind="ExternalInput")
    kh_d = nc.dram_tensor("kh", [D, S], FP16, kind="ExternalInput")
    vr_d = nc.dram_tensor("vr", [HALF, NJ * D], FP16, kind="ExternalInput")
    out_d = nc.dram_tensor("out", [D, S], FP32, kind="ExternalOutput")

    with tile.TileContext(nc) as tc:
        with (
            tc.tile_pool(name="big", bufs=1) as big,
            tc.tile_pool(name="expp", bufs=8) as expp,
            tc.tile_pool(name="daccp", bufs=2) as daccp,
            tc.tile_pool(name="rbsp", bufs=2) as rbsp,
            tc.tile_pool(name="otp", bufs=4) as otp,
            tc.tile_pool(name="ps_s", bufs=2, space="PSUM") as ps_s,
            tc.tile_pool(name="ps_av", bufs=1, space="PSUM") as ps_av,
        ):
            qh = [big.tile([HALF, S], FP16, tag=f"qh{dh}", name=f"qh{dh}")
                  for dh in range(N_DH)]
            kh = [big.tile([HALF, S], FP16, tag=f"kh{dh}", name=f"kh{dh}")
                  for dh in range(N_DH)]
            vr = big.tile([HALF, NJ * D], FP16, tag="vr", name="vr")
            ones_h = big.tile([HALF, HALF], FP16, tag="ones_h", name="ones_h")
            nc.vector.memset(ones_h, 1.0)

            # ---- PE warm-up: flip the HAM clock gate to 8/8 while the
            # first input chunks stream in. ---------------------------------
            wt = ps_s.tile([HALF, QSB], FP32, tag="sp", name="warm")
            for _ in range(N_WARM):
                nc.tensor.matmul(wt[:, :HALF], ones_h, ones_h,
                                 start=True, stop=True)

            # ---- input DMAs, consumption order -----------------------------
            # The two queues (sync HWDGE + gpsimd SWDGE) run in parallel;
            # per-queue order matches first consumption by the main loop.
            # sync: k chunks (all consumed within the first sb); outputs are
            # appended later.
            for a, b in ((0, 512), (512, 1536), (1536, 2560), (2560, 3584),
                         (3584, 4096)):
                for dh in range(N_DH):
                    ds_ = slice(dh * HALF, (dh + 1) * HALF)
                    nc.sync.dma_start(out=kh[dh][:, a:b], in_=kh_d[ds_, a:b])
            # gpsimd: first q super-block, v chunks (consumed from j=0),
            # then the remaining q super-blocks.
            for a, b in ((0, 512), (512, 1024)):
                for dh in range(N_DH):
                    ds_ = slice(dh * HALF, (dh + 1) * HALF)
                    nc.gpsimd.dma_start(out=qh[dh][:, a:b], in_=qh_d[ds_, a:b])
            vchunks = [(0, 512), (512, 1024)] + [
                (1024 * i, 1024 * (i + 1)) for i in range(1, 8)
            ]
            for a, b in vchunks:
                nc.gpsimd.dma_start(out=vr[:, a:b], in_=vr_d[:, a:b])
            for sb in range(1, N_SB):
                for dh in range(N_DH):
                    ds_ = slice(dh * HALF, (dh + 1) * HALF)
                    cs = slice(sb * QSB, (sb + 1) * QSB)
                    nc.gpsimd.dma_start(out=qh[dh][:, cs], in_=qh_d[ds_, cs])

            # ---- software-pipelined main loop ------------------------------
            ets = {}
            daccs = {}
            avs = {}

            def do_s(i):
                sb, j = divmod(i, NJ)
                js = slice(j * HALF, (j + 1) * HALF)
                sp = ps_s.tile([HALF, QSB], FP32, tag="sp", name="sp")
                for c in range(N_QC):
                    cs = slice(sb * QSB + c * QC, sb * QSB + (c + 1) * QC)
                    for dh in range(N_DH):
                        nc.tensor.matmul(sp[:, c * QC:(c + 1) * QC],
                                         kh[dh][:, js], qh[dh][:, cs],
                                         start=(dh == 0), stop=(dh == 1))
                et = expp.tile([HALF, QSB], FP16, tag="et", name="et")
                nc.scalar.activation(
                    et, sp, mybir.ActivationFunctionType.Exp, scale=SCALE
                )
                if j == 0:
                    dacc = daccp.tile([HALF, QSB], FP16, tag="dacc",
                                      name="dacc")
                    nc.vector.tensor_copy(dacc, et)
                    daccs[sb] = dacc
                else:
                    nc.vector.tensor_add(daccs[sb], daccs[sb], et)
                ets[i] = et

            def do_av(i):
                sb, j = divmod(i, NJ)
                if j == 0:
                    avs[sb] = [
                        ps_av.tile([HALF, QSB], FP32, tag=f"av{dh}",
                                   name=f"av{dh}")
                        for dh in range(N_DH)
                    ]
                av = avs[sb]
                rbs = None
                if j == NJ - 1:
                    # Denominator reduce + reciprocal issued BEFORE the last
                    # AV pair: dacc is already complete (its last add trails
                    # exp(j) by one pipeline stage), so the DVE reciprocal
                    # overlaps the final AV matmuls and av[]/rbs are both
                    # ready when the normalization muls start.
                    dacc = daccs.pop(sb)
                    dredt = ps_s.tile([HALF, QSB], FP32, tag="sp",
                                      name="dred")
                    for c in range(N_QC):
                        nc.tensor.matmul(dredt[:, c * QC:(c + 1) * QC],
                                         ones_h,
                                         dacc[:, c * QC:(c + 1) * QC],
                                         start=True, stop=True)
                    rbs = rbsp.tile([HALF, QSB], FP32, tag="rbs", name="rbs")
                    for c in range(N_QC):
                        cs = slice(c * QC, (c + 1) * QC)
                        nc.vector.reciprocal_approx_fast(out=rbs[:, cs],
                                                         in_=dredt[:, cs])
                et = ets.pop(i)
                for dh in range(N_DH):
                    vs = slice(j * D + dh * HALF, j * D + (dh + 1) * HALF)
                    for c in range(N_QC):
                        nc.tensor.matmul(av[dh][:, c * QC:(c + 1) * QC],
                                         vr[:, vs],
                                         et[:, c * QC:(c + 1) * QC],
                                         start=(j == 0), stop=(j == NJ - 1))
                if j == NJ - 1:
                    # Normalize + store in bank-sized chunks so the DVE muls,
                    # the output DMAs, and the next sb's AV matmuls pipeline.
                    for dh in range(N_DH):
                        ot = otp.tile([HALF, QSB], FP32, tag="ot", name="ot")
                        for c in range(N_QC):
                            cs = slice(c * QC, (c + 1) * QC)
                            nc.vector.tensor_mul(ot[:, cs], av[dh][:, cs],
                                                 rbs[:, cs])
                            nc.sync.dma_start(
                                out=out_d[dh * HALF:(dh + 1) * HALF,
                                          sb * QSB + c * QC:
                                          sb * QSB + (c + 1) * QC],
                                in_=ot[:, cs],
                            )
                    avs.pop(sb)

            n_it = N_SB * NJ
            do_s(0)
            do_s(1)
            for i in range(2, n_it):
                do_s(i)
                do_av(i - 2)
            do_av(n_it - 2)
            do_av(n_it - 1)
    nc.finalize()
    return nc


_NC_CACHE = {}


def _get_program():
    if "nc" not in _NC_CACHE:
        _NC_CACHE["nc"] = _build_attention()
    return _NC_CACHE["nc"]


def kernel(queries, keys, values, q_pos, k_pos):
    global LAST_RESULT
    q = np.asarray(queries, dtype=np.float32).reshape(B, D, S)
    k = np.asarray(keys, dtype=np.float32).reshape(B, D, S)
    v = np.asarray(values, dtype=np.float32).reshape(B, D, S)
    qpt = np.asarray(q_pos, np.float32).reshape(S, D).T       # (D, S)
    kpt = np.asarray(k_pos, np.float32).reshape(S, D).T
    qh = (q + qpt[None]).astype(np.float16)                   # (B, D, S)
    kh = (k + kpt[None]).astype(np.float16)
    # v (B, D, S) -> (B, 128, NJ*D): vr[b, p, j*D + d] = v[b, d, j*128 + p]
    vr = np.ascontiguousarray(
        v.reshape(B, D, NJ, HALF).transpose(0, 3, 2, 1).reshape(B, HALF, NJ * D)
    ).astype(np.float16)

    nc = _get_program()
    in_maps = [
        {
            "qh": np.ascontiguousarray(qh[b]),
            "kh": np.ascontiguousarray(kh[b]),
            "vr": vr[b],
        }
        for b in range(B)
    ]
    res = run_bass_kernel_spmd(nc, in_maps, list(range(B)), trace=TRACE)
    LAST_RESULT = res
    out = np.stack([res.results[b]["out"] for b in range(B)])  # (B, D, S)
    return out.reshape(B, D, 64, 64).astype(np.float32)


# revision 7
# speedup vs baseline: 1.5602x; 1.0068x over previous
"""Trainium2 Bass kernel for KerasCrossAttentionModule (B=8, S=4096, D=256).

Sharding: data-parallel over batch B across 8 NeuronCores (1 batch/core).

Host prep (cheap, O(B*S*D)): fold the positional embeddings into q/k
(q + q_pos, k + k_pos), transpose to (D, S), cast fp16; rearrange v to a
key-major layout (128, 32*256) so the whole tensor is one SBUF tile.

Per-core device math (all engines pipelined):
    scoresT[j*128+p, i] = sum_d kh[d, jp] * qh[d, i]     (PE, fp16/fp32 acc)
    E = exp(scale * scoresT)                             (ACT, fp32 -> fp16)
    dacc[p, i] += E[jp, i]                               (DVE partial rowsum)
    av[d, i]  += sum_jp vr[jp, d] * E[jp, i]             (PE accum over j)
    denom = ones^T @ dacc   (M=128 -> full-width, no broadcast needed)
    out[d, i] = av[d, i] * (1 / denom[i])                (DVE approx recip)

The PE instruction stream is software-pipelined: the score matmuls for
iteration i+2 are issued before the AV matmuls of iteration i, so the
exp() latency never stalls the tensor engine.  Inputs stream over the
sync HWDGE queue and the gpsimd SWDGE queue in consumption order (the
ACT queue carries only activations - DMA triggers on it would delay the
first exp by ~20us).  A short warm-up matmul burst flips the PE HAM
clock-gate to full rate while the first input chunks land.

Output DRAM tensor is (D, S) == (DV, H*W), exactly the reference output
layout per batch, so no final transpose is needed.
"""

import os
import sys

import numpy as np

for _p in ("/opt/trn_rl_repo", "/root/.axon_site/_ro/trn_rl_repo"):
    if os.path.isdir(_p) and _p not in sys.path:
        sys.path.insert(0, _p)

import concourse.bass as bass
from concourse import bacc
import concourse.tile as tile
from concourse import mybir
from concourse.bass_utils import run_bass_kernel_spmd

B = 8
D = 256
S = 4096
HALF = 128            # partition-dim tile (D halves / key chunks)
N_DH = D // HALF      # 2 halves of the head dim
QSB = 1024            # query super-block (2 PSUM banks)
QC = 512              # matmul free-dim chunk (1 PSUM bank)
N_QC = QSB // QC
N_SB = S // QSB       # 4 query super-blocks
NJ = S // HALF        # 32 key chunks
SCALE = float(D) ** -0.5
N_WARM = 32           # PE warm-up matmuls (~3.4us @ cold clock)

FP32 = mybir.dt.float32
FP16 = mybir.dt.float16

# Set by test harness to capture a profile; harness-default is plain run.
TRACE = False
LAST_RESULT = None


def _build_attention():
    """One-core program; identical on all 8 cores (pure data parallel)."""
    nc = bacc.Bacc("TRN2")
    qh_d = nc.dram_tensor("qh", [D, S], FP16, kind="ExternalInput")
    kh_d = nc.dram_tensor("kh", [D, S], FP16, kind="ExternalInput")
    vr_d = nc.dram_tensor("vr", [HALF, NJ * D], FP16, kind="ExternalInput")
    out_d = nc.dram_tensor("out", [D, S], FP32, kind="ExternalOutput")

    with tile.TileContext(nc) as tc:
        with (
            tc.tile_pool(name="big", bufs=1) as big,
            tc.tile_pool(name="expp", bufs=8) as expp,
            tc.tile_pool(name="daccp", bufs=2) as daccp,
            tc.tile_pool(name="rbsp", bufs=2) as rbsp,
            tc.tile_pool(name="otp", bufs=4) as otp,
            tc.tile_pool(name="ps_s", bufs=2, space="PSUM") as ps_s,
            tc.tile_pool(name="ps_av", bufs=1, space="PSUM") as ps_av,
        ):
            qh = [big.tile([HALF, S], FP16, tag=f"qh{dh}", name=f"qh{dh}")
                  for dh in range(N_DH)]
            kh = [big.tile([HALF, S], FP16, tag=f"kh{dh}", name=f"kh{dh}")
                  for dh in range(N_DH)]
            vr = big.tile([HALF, NJ * D], FP16, tag="vr", name="vr")
            ones_h = big.tile([HALF, HALF], FP16, tag="ones_h", name="ones_h")
            nc.vector.memset(ones_h, 1.0)

            # ---- PE warm-up: flip the HAM clock gate to 8/8 while the
            # first input chunks stream in. ---------------------------------
            wt = ps_s.tile([HALF, QSB], FP32, tag="sp", name="warm")
            for _ in range(N_WARM):
                nc.tensor.matmul(wt[:, :HALF], ones_h, ones_h,
                                 start=True, stop=True)

            # ---- input DMAs, consumption order -----------------------------
            # The two queues (sync HWDGE + gpsimd SWDGE) run in parallel;
            # per-queue order matches first consumption by the main loop.
            # sync: k chunks (all consumed within the first sb); outputs are
            # appended later.
            for dh in range(N_DH):
                ds_ = slice(dh * HALF, (dh + 1) * HALF)
                nc.sync.dma_start(out=kh[dh][:, 0:512], in_=kh_d[ds_, 0:512])
            for dh in range(N_DH):
                ds_ = slice(dh * HALF, (dh + 1) * HALF)
                nc.sync.dma_start(out=qh[dh][:, 512:1024],
                                  in_=qh_d[ds_, 512:1024])
            for a, b in ((512, 1536), (1536, 2560), (2560, 3584),
                         (3584, 4096)):
                for dh in range(N_DH):
                    ds_ = slice(dh * HALF, (dh + 1) * HALF)
                    nc.sync.dma_start(out=kh[dh][:, a:b], in_=kh_d[ds_, a:b])
            # gpsimd: first half of the first q super-block, v chunks
            # (consumed from j=0), then the remaining q super-blocks.
            for dh in range(N_DH):
                ds_ = slice(dh * HALF, (dh + 1) * HALF)
                nc.gpsimd.dma_start(out=qh[dh][:, 0:512], in_=qh_d[ds_, 0:512])
            vchunks = [(0, 512), (512, 1024)] + [
                (1024 * i, 1024 * (i + 1)) for i in range(1, 8)
            ]
            for a, b in vchunks:
                nc.gpsimd.dma_start(out=vr[:, a:b], in_=vr_d[:, a:b])
            for sb in range(1, N_SB):
                for dh in range(N_DH):
                    ds_ = slice(dh * HALF, (dh + 1) * HALF)
                    cs = slice(sb * QSB, (sb + 1) * QSB)
                    nc.gpsimd.dma_start(out=qh[dh][:, cs], in_=qh_d[ds_, cs])

            # ---- software-pipelined main loop ------------------------------
            ets = {}
            daccs = {}
            avs = {}

            def do_s(i):
                sb, j = divmod(i, NJ)
                js = slice(j * HALF, (j + 1) * HALF)
                sp = ps_s.tile([HALF, QSB], FP32, tag="sp", name="sp")
                for c in range(N_QC):
                    cs = slice(sb * QSB + c * QC, sb * QSB + (c + 1) * QC)
                    for dh in range(N_DH):
                        nc.tensor.matmul(sp[:, c * QC:(c + 1) * QC],
                                         kh[dh][:, js], qh[dh][:, cs],
                                         start=(dh == 0), stop=(dh == 1))
                et = expp.tile([HALF, QSB], FP16, tag="et", name="et")
                nc.scalar.activation(
                    et, sp, mybir.ActivationFunctionType.Exp, scale=SCALE
                )
                if j == 0:
                    dacc = daccp.tile([HALF, QSB], FP16, tag="dacc",
                                      name="dacc")
                    nc.vector.tensor_copy(dacc, et)
                    daccs[sb] = dacc
                else:
                    nc.vector.tensor_add(daccs[sb], daccs[sb], et)
                ets[i] = et

            def do_av(i):
                sb, j = divmod(i, NJ)
                if j == 0:
                    avs[sb] = [
                        ps_av.tile([HALF, QSB], FP32, tag=f"av{dh}",
                                   name=f"av{dh}")
                        for dh in range(N_DH)
                    ]
                av = avs[sb]
                rbs = None
                if j == NJ - 1:
                    # Denominator reduce + reciprocal issued BEFORE the last
                    # AV pair: dacc is already complete (its last add trails
                    # exp(j) by one pipeline stage), so the DVE reciprocal
                    # overlaps the final AV matmuls and av[]/rbs are both
                    # ready when the normalization muls start.
                    dacc = daccs.pop(sb)
                    dredt = ps_s.tile([HALF, QSB], FP32, tag="sp",
                                      name="dred")
                    for c in range(N_QC):
                        nc.tensor.matmul(dredt[:, c * QC:(c + 1) * QC],
                                         ones_h,
                                         dacc[:, c * QC:(c + 1) * QC],
                                         start=True, stop=True)
                    rbs = rbsp.tile([HALF, QSB], FP32, tag="rbs", name="rbs")
                    for c in range(N_QC):
                        cs = slice(c * QC, (c + 1) * QC)
                        nc.vector.reciprocal_approx_fast(out=rbs[:, cs],
                                                         in_=dredt[:, cs])
                et = ets.pop(i)
                for dh in range(N_DH):
                    vs = slice(j * D + dh * HALF, j * D + (dh + 1) * HALF)
                    for c in range(N_QC):
                        nc.tensor.matmul(av[dh][:, c * QC:(c + 1) * QC],
                                         vr[:, vs],
                                         et[:, c * QC:(c + 1) * QC],
                                         start=(j == 0), stop=(j == NJ - 1))
                if j == NJ - 1:
                    # Normalize + store in bank-sized chunks so the DVE muls,
                    # the output DMAs, and the next sb's AV matmuls pipeline.
                    for dh in range(N_DH):
                        ot = otp.tile([HALF, QSB], FP32, tag="ot", name="ot")
                        for c in range(N_QC):
                            cs = slice(c * QC, (c + 1) * QC)
                            nc.vector.tensor_mul(ot[:, cs], av[dh][:, cs],
                                                 rbs[:, cs])
                            nc.sync.dma_start(
                                out=out_d[dh * HALF:(dh + 1) * HALF,
                                          sb * QSB + c * QC:
                                          sb * QSB + (c + 1) * QC],
                                in_=ot[:, cs],
                            )
                    avs.pop(sb)

            n_it = N_SB * NJ
            do_s(0)
            do_s(1)
            for i in range(2, n_it):
                do_s(i)
                do_av(i - 2)
            do_av(n_it - 2)
            do_av(n_it - 1)
    nc.finalize()
    return nc


_NC_CACHE = {}


def _get_program():
    if "nc" not in _NC_CACHE:
        _NC_CACHE["nc"] = _build_attention()
    return _NC_CACHE["nc"]


def kernel(queries, keys, values, q_pos, k_pos):
    global LAST_RESULT
    q = np.asarray(queries, dtype=np.float32).reshape(B, D, S)
    k = np.asarray(keys, dtype=np.float32).reshape(B, D, S)
    v = np.asarray(values, dtype=np.float32).reshape(B, D, S)
    qpt = np.asarray(q_pos, np.float32).reshape(S, D).T       # (D, S)
    kpt = np.asarray(k_pos, np.float32).reshape(S, D).T
    qh = (q + qpt[None]).astype(np.float16)                   # (B, D, S)
    kh = (k + kpt[None]).astype(np.float16)
    # v (B, D, S) -> (B, 128, NJ*D): vr[b, p, j*D + d] = v[b, d, j*128 + p]
    vr = np.ascontiguousarray(
        v.reshape(B, D, NJ, HALF).transpose(0, 3, 2, 1).reshape(B, HALF, NJ * D)
    ).astype(np.float16)

    nc = _get_program()
    in_maps = [
        {
            "qh": np.ascontiguousarray(qh[b]),
            "kh": np.ascontiguousarray(kh[b]),
            "vr": vr[b],
        }
        for b in range(B)
    ]
    res = run_bass_kernel_spmd(nc, in_maps, list(range(B)), trace=TRACE)
    LAST_RESULT = res
    out = np.stack([res.results[b]["out"] for b in range(B)])  # (B, D, S)
    return out.reshape(B, D, 64, 64).astype(np.float32)


# revision 8
# speedup vs baseline: 1.5707x; 1.0067x over previous
"""Trainium2 Bass kernel for KerasCrossAttentionModule (B=8, S=4096, D=256).

Sharding: data-parallel over batch B across 8 NeuronCores (1 batch/core).

Host prep (cheap, O(B*S*D)): fold the positional embeddings into q/k
(q + q_pos, k + k_pos), transpose to (D, S), cast fp16; rearrange v to a
key-major layout (128, 32*256) so the whole tensor is one SBUF tile.

Per-core device math (all engines pipelined):
    scoresT[j*128+p, i] = sum_d kh[d, jp] * qh[d, i]     (PE, fp16/fp32 acc)
    E = exp(scale * scoresT)                             (ACT, fp32 -> fp16)
    dacc[p, i] += E[jp, i]                               (DVE partial rowsum)
    av[d, i]  += sum_jp vr[jp, d] * E[jp, i]             (PE accum over j)
    denom = ones^T @ dacc   (M=128 -> full-width, no broadcast needed)
    out[d, i] = av[d, i] * (1 / denom[i])                (DVE approx recip)

The PE instruction stream is software-pipelined: the score matmuls for
iteration i+2 are issued before the AV matmuls of iteration i, so the
exp() latency never stalls the tensor engine.  Inputs stream over the
sync HWDGE queue and the gpsimd SWDGE queue in consumption order (the
ACT queue carries only activations - DMA triggers on it would delay the
first exp by ~20us).  A short warm-up matmul burst flips the PE HAM
clock-gate to full rate while the first input chunks land.

Output DRAM tensor is (D, S) == (DV, H*W), exactly the reference output
layout per batch, so no final transpose is needed.
"""

import os
import sys

import numpy as np

for _p in ("/opt/trn_rl_repo", "/root/.axon_site/_ro/trn_rl_repo"):
    if os.path.isdir(_p) and _p not in sys.path:
        sys.path.insert(0, _p)

import concourse.bass as bass
from concourse import bacc
import concourse.tile as tile
from concourse import mybir
from concourse.bass_utils import run_bass_kernel_spmd

B = 8
D = 256
S = 4096
HALF = 128            # partition-dim tile (D halves / key chunks)
N_DH = D // HALF      # 2 halves of the head dim
QSB = 1024            # query super-block (2 PSUM banks)
QC = 512              # matmul free-dim chunk (1 PSUM bank)
N_QC = QSB // QC
N_SB = S // QSB       # 4 query super-blocks
NJ = S // HALF        # 32 key chunks
SCALE = float(D) ** -0.5
N_WARM = 64           # PE warm-up matmuls (~5.2us: HAM flips warm at ~3.4us,
                      # the rest run 2x faster and bridge to DMA data-arrival)

FP32 = mybir.dt.float32
FP16 = mybir.dt.float16

# Set by test harness to capture a profile; harness-default is plain run.
TRACE = False
LAST_RESULT = None


def _build_attention():
    """One-core program; identical on all 8 cores (pure data parallel)."""
    nc = bacc.Bacc("TRN2")
    qh_d = nc.dram_tensor("qh", [D, S], FP16, kind="ExternalInput")
    kh_d = nc.dram_tensor("kh", [D, S], FP16, kind="ExternalInput")
    vr_d = nc.dram_tensor("vr", [HALF, NJ * D], FP16, kind="ExternalInput")
    out_d = nc.dram_tensor("out", [D, S], FP32, kind="ExternalOutput")

    with tile.TileContext(nc) as tc:
        with (
            tc.tile_pool(name="big", bufs=1) as big,
            tc.tile_pool(name="expp", bufs=8) as expp,
            tc.tile_pool(name="daccp", bufs=2) as daccp,
            tc.tile_pool(name="rbsp", bufs=2) as rbsp,
            tc.tile_pool(name="otp", bufs=4) as otp,
            tc.tile_pool(name="ps_s", bufs=2, space="PSUM") as ps_s,
            tc.tile_pool(name="ps_av", bufs=1, space="PSUM") as ps_av,
        ):
            qh = [big.tile([HALF, S], FP16, tag=f"qh{dh}", name=f"qh{dh}")
                  for dh in range(N_DH)]
            kh = [big.tile([HALF, S], FP16, tag=f"kh{dh}", name=f"kh{dh}")
                  for dh in range(N_DH)]
            vr = big.tile([HALF, NJ * D], FP16, tag="vr", name="vr")
            ones_h = big.tile([HALF, HALF], FP16, tag="ones_h", name="ones_h")
            nc.vector.memset(ones_h, 1.0)

            # ---- PE warm-up: flip the HAM clock gate to 8/8 while the
            # first input chunks stream in. ---------------------------------
            wt = ps_s.tile([HALF, QSB], FP32, tag="sp", name="warm")
            for _ in range(N_WARM):
                nc.tensor.matmul(wt[:, :HALF], ones_h, ones_h,
                                 start=True, stop=True)

            # ---- input DMAs, consumption order -----------------------------
            # The two queues (sync HWDGE + gpsimd SWDGE) run in parallel;
            # per-queue order matches first consumption by the main loop.
            # sync: k chunks (all consumed within the first sb); outputs are
            # appended later.
            for dh in range(N_DH):
                ds_ = slice(dh * HALF, (dh + 1) * HALF)
                nc.sync.dma_start(out=kh[dh][:, 0:512], in_=kh_d[ds_, 0:512])
            for dh in range(N_DH):
                ds_ = slice(dh * HALF, (dh + 1) * HALF)
                nc.sync.dma_start(out=qh[dh][:, 512:1024],
                                  in_=qh_d[ds_, 512:1024])
            for a, b in ((512, 1536), (1536, 2560), (2560, 3584),
                         (3584, 4096)):
                for dh in range(N_DH):
                    ds_ = slice(dh * HALF, (dh + 1) * HALF)
                    nc.sync.dma_start(out=kh[dh][:, a:b], in_=kh_d[ds_, a:b])
            # gpsimd: first half of the first q super-block, v chunks
            # (consumed from j=0), then the remaining q super-blocks.
            for dh in range(N_DH):
                ds_ = slice(dh * HALF, (dh + 1) * HALF)
                nc.gpsimd.dma_start(out=qh[dh][:, 0:512], in_=qh_d[ds_, 0:512])
            vchunks = [(0, 512), (512, 1024)] + [
                (1024 * i, 1024 * (i + 1)) for i in range(1, 8)
            ]
            for a, b in vchunks:
                nc.gpsimd.dma_start(out=vr[:, a:b], in_=vr_d[:, a:b])
            for sb in range(1, N_SB):
                for dh in range(N_DH):
                    ds_ = slice(dh * HALF, (dh + 1) * HALF)
                    cs = slice(sb * QSB, (sb + 1) * QSB)
                    nc.gpsimd.dma_start(out=qh[dh][:, cs], in_=qh_d[ds_, cs])

            # ---- software-pipelined main loop ------------------------------
            ets = {}
            daccs = {}
            avs = {}

            def do_s(i):
                sb, j = divmod(i, NJ)
                js = slice(j * HALF, (j + 1) * HALF)
                sp = ps_s.tile([HALF, QSB], FP32, tag="sp", name="sp")
                for c in range(N_QC):
                    cs = slice(sb * QSB + c * QC, sb * QSB + (c + 1) * QC)
                    for dh in range(N_DH):
                        nc.tensor.matmul(sp[:, c * QC:(c + 1) * QC],
                                         kh[dh][:, js], qh[dh][:, cs],
                                         start=(dh == 0), stop=(dh == 1))
                et = expp.tile([HALF, QSB], FP16, tag="et", name="et")
                nc.scalar.activation(
                    et, sp, mybir.ActivationFunctionType.Exp, scale=SCALE
                )
                if j == 0:
                    dacc = daccp.tile([HALF, QSB], FP16, tag="dacc",
                                      name="dacc")
                    nc.vector.tensor_copy(dacc, et)
                    daccs[sb] = dacc
                else:
                    nc.vector.tensor_add(daccs[sb], daccs[sb], et)
                ets[i] = et

            def do_av(i):
                sb, j = divmod(i, NJ)
                if j == 0:
                    avs[sb] = [
                        ps_av.tile([HALF, QSB], FP32, tag=f"av{dh}",
                                   name=f"av{dh}")
                        for dh in range(N_DH)
                    ]
                av = avs[sb]
                rbs = None
                if j == NJ - 1:
                    # Denominator reduce + reciprocal issued BEFORE the last
                    # AV pair: dacc is already complete (its last add trails
                    # exp(j) by one pipeline stage), so the DVE reciprocal
                    # overlaps the final AV matmuls and av[]/rbs are both
                    # ready when the normalization muls start.
                    dacc = daccs.pop(sb)
                    dredt = ps_s.tile([HALF, QSB], FP32, tag="sp",
                                      name="dred")
                    for c in range(N_QC):
                        nc.tensor.matmul(dredt[:, c * QC:(c + 1) * QC],
                                         ones_h,
                                         dacc[:, c * QC:(c + 1) * QC],
                                         start=True, stop=True)
                    rbs = rbsp.tile([HALF, QSB], FP32, tag="rbs", name="rbs")
                    for c in range(N_QC):
                        cs = slice(c * QC, (c + 1) * QC)
                        nc.vector.reciprocal_approx_fast(out=rbs[:, cs],
                                                         in_=dredt[:, cs])
                et = ets.pop(i)
                for dh in range(N_DH):
                    vs = slice(j * D + dh * HALF, j * D + (dh + 1) * HALF)
                    for c in range(N_QC):
                        nc.tensor.matmul(av[dh][:, c * QC:(c + 1) * QC],
                                         vr[:, vs],
                                         et[:, c * QC:(c + 1) * QC],
                                         start=(j == 0), stop=(j == NJ - 1))
                if j == NJ - 1:
                    # Normalize + store in bank-sized chunks so the DVE muls,
                    # the output DMAs, and the next sb's AV matmuls pipeline.
                    for dh in range(N_DH):
                        ot = otp.tile([HALF, QSB], FP32, tag="ot", name="ot")
                        for c in range(N_QC):
                            cs = slice(c * QC, (c + 1) * QC)
                            nc.vector.tensor_mul(ot[:, cs], av[dh][:, cs],
                                                 rbs[:, cs])
                            nc.sync.dma_start(
                                out=out_d[dh * HALF:(dh + 1) * HALF,
                                          sb * QSB + c * QC:
                                          sb * QSB + (c + 1) * QC],
                                in_=ot[:, cs],
                            )
                    avs.pop(sb)

            n_it = N_SB * NJ
            do_s(0)
            do_s(1)
            for i in range(2, n_it):
                do_s(i)
                do_av(i - 2)
            do_av(n_it - 2)
            do_av(n_it - 1)
    nc.finalize()
    return nc


_NC_CACHE = {}


def _get_program():
    if "nc" not in _NC_CACHE:
        _NC_CACHE["nc"] = _build_attention()
    return _NC_CACHE["nc"]


def kernel(queries, keys, values, q_pos, k_pos):
    global LAST_RESULT
    q = np.asarray(queries, dtype=np.float32).reshape(B, D, S)
    k = np.asarray(keys, dtype=np.float32).reshape(B, D, S)
    v = np.asarray(values, dtype=np.float32).reshape(B, D, S)
    qpt = np.asarray(q_pos, np.float32).reshape(S, D).T       # (D, S)
    kpt = np.asarray(k_pos, np.float32).reshape(S, D).T
    qh = (q + qpt[None]).astype(np.float16)                   # (B, D, S)
    kh = (k + kpt[None]).astype(np.float16)
    # v (B, D, S) -> (B, 128, NJ*D): vr[b, p, j*D + d] = v[b, d, j*128 + p]
    vr = np.ascontiguousarray(
        v.reshape(B, D, NJ, HALF).transpose(0, 3, 2, 1).reshape(B, HALF, NJ * D)
    ).astype(np.float16)

    nc = _get_program()
    in_maps = [
        {
            "qh": np.ascontiguousarray(qh[b]),
            "kh": np.ascontiguousarray(kh[b]),
            "vr": vr[b],
        }
        for b in range(B)
    ]
    res = run_bass_kernel_spmd(nc, in_maps, list(range(B)), trace=TRACE)
    LAST_RESULT = res
    out = np.stack([res.results[b]["out"] for b in range(B)])  # (B, D, S)
    return out.reshape(B, D, 64, 64).astype(np.float32)
